# revision 24
# baseline (speedup 1.0000x reference)
"""Trainium2 Bass kernel: batched locally-weighted ridge regression.

Per test point t: K[t,n] = exp(-|xte_t - xtr_n|^2 / (2 ls^2));
  A_t = Xtild^T diag(K[t]) Xtild + REG*I ; b_t = Xtild^T (K[t] * Y)
  ypred_t = xtild_t . A_t^{-1} b_t
Sharding: data-parallel over the 4096 test points -> 8 cores x 512.

On-device math uses a scaled kernel K'[t,n] = exp((S[n,t] - sn[n]/2) * c2)
(c2 = 1/ls^2), i.e. the exp(-st*c2/2) per-test factor is dropped; this
rescales A_t and b_t identically, so beta is preserved by using a
per-test ridge REG_t = REG * exp(st*c2/2).

Pipeline layout (per core; engines run concurrently):
  - Input DMA spread across SP/DVE/Pool/ACT queues (the cost model charges
    transfers to the issuing engine), zz ordered chunk-major so the XWX
    accumulation can chase the arrivals.
  - PE: warmup matmuls (clock ramp), then gram for the test-half 0,
    XWX for t-tile 0 (range-interleaved, chasing the exp chain on ACT),
    gram half 1, XWX t1..t3.  All matmuls f32r, 1 cycle/row (N>=256).
  - ACT: exp of the gram chunks at [P, 2, 256] grain, then the PSUM
    evacuations assembling [A | b | xtt] per t-tile.
  - Solve: LDLt-style symmetric elimination of 4 units of 128 systems,
    emitted with a staggered interleave so units pipeline.  Per step:
    Pool normalize_recip produces ps = pivot_row * (1/d) AND stores 1/d
    on the diagonal in one op; a Pool pivot-row lookahead (row k+1)
    keeps the serial chain engine-local; DVE updates a band of rows
    below the pivot as one rect; Pool covers the remaining rows with
    column-panel rectangles.  No back-substitution: with A = L D L^T,
    ypred = sum_k invp_k * (L^-1 b)_k * (L^-1 xtt)_k.
"""

import numpy as np

import concourse.bacc as bacc
import concourse.mybir as mybir
from concourse.bass import ts
from concourse.bass_utils import run_bass_kernel_spmd
from concourse.tile import TileContext

F32 = mybir.dt.float32
F32R = mybir.dt.float32r
P = 128
N_TRAIN = 2048
D = 31
DP = 32          # 1 + D
W = 34           # DP + two rhs columns: b (col 32), xtt (col 33)
N_TEST = 4096
NCORES = 8
TS = N_TEST // NCORES   # 512 test points per core
NT = TS // P            # 4 t-tiles
NK = N_TRAIN // P       # 16 train chunks
NZ = 672                # 4 e-strips of the upper pairs (64+128+192+256) + 32 (x*y)
REG = 1e-6
TH = 256                # test-half width for the gram/exp pipeline

N_WARMUP = 4            # PE clock-ramp warmup matmuls
UNITS = ((0, 1), (1, 3), (3, 4))   # solve units as t-tile ranges
OFFS = (0, 17, 27)                 # per-unit emission offsets (in steps)
BSMAX = 16              # max DVE band rows

# engine cost model (ns) used to pick the DVE band size per step
_POOL_TAX, _POOL_EL = 8.0, 0.8333
_DVE_TAX, _DVE_EL = 90.0, 1.0417


def _panels_cost(r2: int, k: int, ns: int = 1) -> float:
    w_ = 0.0
    for p in range(r2 // 8, 4):
        re_ = 8 * p + 8
        if re_ <= r2:
            continue
        cs = max(8 * p, k + 1)
        ce = W if p == 3 else re_
        w_ += 2 * (_POOL_TAX + ns * (re_ - r2) * (ce - cs) * _POOL_EL)
    return w_


def _band(k: int, ns: int = 1) -> int:
    """DVE band rows at step k (rows k+2 .. k+2+bs); Pool panels cover the
    rest.  Chosen to balance the two engines' per-step busy time."""
    m1 = 30 - k
    if m1 <= 0:
        return 0
    w = W - 1 - k
    nrow = 2 if ns == 1 else 3
    base_pool = nrow * (_POOL_TAX + ns * w * _POOL_EL)
    best, best_cost = 0, None
    for bs in range(0, min(BSMAX, m1) + 1):
        dve = 2 * (_DVE_TAX + ns * bs * w * _DVE_EL) if bs else 0.0
        pool = base_pool + _panels_cost(k + 2 + bs, k, ns)
        cost = max(dve, pool)
        if best_cost is None or cost < best_cost:
            best, best_cost = bs, cost
    return best


def _build_nc(c2: float):
    """Build the single-core Bass program (SPMD across 8 cores)."""
    nc = bacc.Bacc(trn_type="TRN2")

    # transposed features packed for 4-way row-group gram matmuls:
    # [32g+d, cc*128+p] = Xtrain[(4*cc+g)*128+p, d]; cols 512: = XtestT x4
    xT_d = nc.dram_tensor("xT", [P, 4 * P + TS], F32R, kind="ExternalInput")
    zz_d = nc.dram_tensor("zz", [P, NK * NZ], F32R, kind="ExternalInput")
    regt_d = nc.dram_tensor("regt", [P, NT], F32, kind="ExternalInput")
    xtt_d = nc.dram_tensor("xtt", [P, NT * DP], F32, kind="ExternalInput")
    out_d = nc.dram_tensor("ypred", [TS, 1], F32, kind="ExternalOutput")

    with TileContext(nc) as tc:
        with (
            tc.tile_pool(name="sb", bufs=1) as sb,
            tc.tile_pool(name="pgram", bufs=2, space="PSUM") as pgram,
            tc.tile_pool(name="pxwx", bufs=4, space="PSUM") as pxwx,
        ):
            # ---- input loads spread across engine queues; chunk-major zz so
            # the XWX accumulation can chase the arrivals.
            xT = sb.tile([P, 4 * P + TS], F32R)
            zz = sb.tile([P, NK, NZ], F32R)
            zr = zz[:].rearrange("p c z -> p (c z)")
            regt = sb.tile([P, NT], F32)
            xtt = sb.tile([P, NT, DP], F32)

            def zslice(c):
                return (zr[:, ts(c, NZ)], zz_d[:, ts(c, NZ)])

            # SP: xT (needed first), xtt, zz odd chunks, regt
            nc.sync.dma_start(xT[:, 512:], xT_d[:, 512:])
            nc.sync.dma_start(xT[:, 0:P], xT_d[:, 0:P])
            nc.sync.dma_start(xT[:, P:512], xT_d[:, P:512])
            for c in (1, 3, 5, 7, 9, 11, 14):
                nc.sync.dma_start(*zslice(c))
            nc.sync.dma_start(
                xtt, xtt_d.rearrange("p (t d) -> p t d", t=NT)
            )
            nc.sync.dma_start(regt, regt_d[:, :])
            wu = sb.tile([P, 512], F32R)
            nc.vector.memset(wu, 1.0)
            # Pool: zz even chunks (Pool is solve-idle until ~10us)
            for c in (0, 2, 4, 6, 8, 10, 12):
                nc.gpsimd.dma_start(*zslice(c))
            # ga sub-diagonal zeroing on idle-early DVE (disjoint from the
            # evac strips, so it can run before the XWX finishes)
            # ACT: two slices in its idle window before the first gram
            # pair lands (~2.6us); exps are not delayed
            nc.scalar.dma_start(*zslice(13))
            nc.scalar.dma_start(*zslice(15))

            # ---- PE warmup: ramp the clock during the DMAs ----
            for _ in range(N_WARMUP):
                wps = pxwx.tile([P, 512], F32, tag="px")
                nc.tensor.matmul(wps, wu[0:8, 0:128], wu[0:8, :],
                                 start=True, stop=True)

            # ---- gram S' = S - sn/2 and K' = exp(S'*c2), by test-half;
            # the -sn/2 bias rides in row 31 of each packed xT group ----
            kp = sb.tile([P, NK, TS], F32R)

            def gram_pair(h, cp):
                sg = pgram.tile([P, 2, TH], F32, tag="sg")
                for i in range(2):
                    c = 2 * cp + i
                    cc, g = c // 4, c % 4
                    nc.tensor.matmul(
                        sg[:, i, :],
                        xT[32 * g:32 * g + DP, ts(cc, P)],
                        xT[32 * g:32 * g + DP,
                           4 * P + h * TH:4 * P + (h + 1) * TH],
                        start=True, stop=True,
                        tile_position=(32 * g, 0),
                    )
                nc.scalar.activation(
                    kp[:, 2 * cp:2 * cp + 2, h * TH:(h + 1) * TH], sg,
                    mybir.ActivationFunctionType.Exp,
                    scale=c2,
                )

            # ---- per t-tile: XWX/XWy matmuls (range-interleaved so both
            # PSUM accumulators chase the exp chain), then ACT evacuation ----
            ga = sb.tile([P, NT, DP, W], F32)
            ga_diag = ga[:].rearrange("p b r c -> p b (r c)")[:, :, ::W + 1]
            # zero the uninitialized sub-diagonal cells the band rects read
            # (row-block > col-block: not covered by any evac strip); runs on
            # idle-early DVE, disjoint from everything
            for h in range(NT):
                nc.vector.memset(ga[:, h, 8:32, 0:8], 0.0)
                nc.vector.memset(ga[:, h, 16:32, 8:16], 0.0)
                nc.vector.memset(ga[:, h, 24:32, 16:24], 0.0)

            def xwx_t(t, gram_b=False):
                pxa = pxwx.tile([P, 512], F32, tag="px")
                pxb = pxwx.tile([P, 512], F32, tag="px")
                corder = ((0, 1, 2, 3, 4, 6, 5, 8, 7, 10, 13, 9, 14, 15,
                           12, 11) if t == 0 else range(NK))
                for ci, c in enumerate(corder):
                    nc.tensor.matmul(
                        pxa[:, 0:384],
                        kp[:, c, ts(t, P)], zz[:, c, 0:384],
                        start=(ci == 0), stop=(ci == NK - 1),
                    )
                    nc.tensor.matmul(
                        pxb[:, 0:NZ - 384],
                        kp[:, c, ts(t, P)], zz[:, c, 384:NZ],
                        start=(ci == 0), stop=(ci == NK - 1),
                    )
                    if gram_b and ci % 2 == 1:
                        # half-1 gram pairs soak the gate stalls of t0's
                        # accumulation; their exps queue behind exps-A on ACT
                        gram_pair(1, ci // 2)
                # evacuation: t0 on (idle) DVE so the solve can start without
                # queuing behind ACT's exp chain; t1..t3 on ACT
                def ecopy(dst, src):
                    if t == 0:
                        nc.vector.tensor_scalar_mul(dst, src, 1.0)
                    else:
                        nc.scalar.copy(dst, src)
                off = 0
                for s in range(3):
                    rs = 8 * s + 8
                    ecopy(
                        ga[:, t, 0:rs, 8 * s:8 * s + 8],
                        pxa[:, off:off + 8 * rs].rearrange(
                            "p (r c) -> p r c", r=rs),
                    )
                    off += 8 * rs
                # strip 3 + rhs column + test design row from range B
                ecopy(
                    ga[:, t, :, 24:32],
                    pxb[:, 0:256].rearrange("p (r c) -> p r c", r=DP),
                )
                ecopy(ga[:, t, :, DP], pxb[:, 256:256 + DP])
                ecopy(ga[:, t, :, DP + 1], xtt[:, t])
                # per-test ridge on the diagonal (t0: DVE; rest: ACT)
                if t == 0:
                    nc.vector.tensor_add(
                        ga_diag[:, t], ga_diag[:, t],
                        regt[:, t, None].broadcast_to([P, DP]),
                    )
                else:
                    nc.scalar.add(ga_diag[:, t], ga_diag[:, t],
                                  regt[:, t:t + 1])

            for cp in range(NK // 2):
                gram_pair(0, cp)
            xwx_t(0, gram_b=True)
            xwx_t(1)
            xwx_t(2)
            xwx_t(3)

            # ---- solve: symmetric (LDLt-style) elimination over UNITS of
            # 1-2 adjacent t-tiles (a multi-tile unit eliminates its tiles
            # in lockstep inside the same instructions: same op count, n x
            # the elements, so per-step taxes and serial chain amortize).
            # A[i,k] = A[k,i], so the update is upd[i,j] =
            # (A[k,i]/d)*A[k,j] = A[k,i]*ps[j]; only the upper triangle +
            # rhs is ever read.
            ps = sb.tile([P, NT, W], F32)
            yp = sb.tile([P, NT], F32)
            tbr = sb.tile([P, NT, W], F32)
            tbD = sb.tile([P, NT, BSMAX, W], F32)
            tbP = sb.tile([P, NT, DP, 12], F32)
            prod = sb.tile([P, NT, DP], F32)

            def step(u0, u1, k):
                if k == DP - 1:
                    return
                ns = u1 - u0
                w = W - 1 - k              # cols k+1..33 (incl rhs)
                # pivot: ps = -row_k[k+1:] / d  (negated so every downstream
                # update is an add; fused div+negate for 1-tile units)
                if ns == 1:
                    nc.gpsimd.tensor_scalar(
                        ps[:, u0, :w], ga[:, u0, k, k + 1:W],
                        ga[:, u0, k, k:k + 1], -1.0,
                        op0=mybir.AluOpType.divide,
                        op1=mybir.AluOpType.mult,
                    )
                    # lookahead row k+1, fused: ga_row += ps * A[k,k+1]
                    # (Pool-local chain: div -> row -> next div)
                    nc.gpsimd.scalar_tensor_tensor(
                        ga[:, u0, k + 1, k + 1:W], ps[:, u0, :w],
                        ga[:, u0, k, k + 1:k + 2], ga[:, u0, k + 1, k + 1:W],
                        op0=mybir.AluOpType.mult, op1=mybir.AluOpType.add,
                    )
                else:
                    # positive ps via broadcast divide; downstream ops
                    # subtract instead of add for this unit
                    nc.gpsimd.tensor_tensor(
                        ps[:, u0:u1, :w], ga[:, u0:u1, k, k + 1:W],
                        ga[:, u0:u1, k, k:k + 1].broadcast_to([P, ns, w]),
                        op=mybir.AluOpType.divide,
                    )
                    nc.gpsimd.tensor_mul(
                        tbr[:, u0:u1, :w], ps[:, u0:u1, :w],
                        ga[:, u0:u1, k, k + 1:k + 2]
                        .broadcast_to([P, ns, w]),
                    )
                    nc.gpsimd.tensor_sub(
                        ga[:, u0:u1, k + 1, k + 1:W],
                        ga[:, u0:u1, k + 1, k + 1:W], tbr[:, u0:u1, :w])
                acc = (mybir.AluOpType.add if ns == 1
                       else mybir.AluOpType.subtract)
                bs = _band(k, ns)
                r2 = k + 2 + bs
                if bs > 0:
                    # DVE band: rows k+2..r2, all cols k+1..33
                    nc.vector.tensor_mul(
                        tbD[:, u0:u1, :bs, :w],
                        ga[:, u0:u1, k, k + 2:r2, None]
                        .broadcast_to([P, ns, bs, w]),
                        ps[:, u0:u1, None, :w].broadcast_to([P, ns, bs, w]),
                    )
                    nc.vector.tensor_tensor(
                        ga[:, u0:u1, k + 2:r2, k + 1:W],
                        ga[:, u0:u1, k + 2:r2, k + 1:W],
                        tbD[:, u0:u1, :bs, :w], op=acc,
                    )
                # remaining rows r2..31 by column panels (Pool)
                for p in range(r2 // 8, 4):
                    re_ = 8 * p + 8
                    if re_ <= r2:
                        continue
                    cs = max(8 * p, k + 1)
                    ce = W if p == 3 else re_
                    mr, wc = re_ - r2, ce - cs
                    nc.gpsimd.tensor_mul(
                        tbP[:, u0:u1, :mr, :wc],
                        ga[:, u0:u1, k, r2:re_, None]
                        .broadcast_to([P, ns, mr, wc]),
                        ps[:, u0:u1, None, cs - k - 1:ce - k - 1]
                        .broadcast_to([P, ns, mr, wc]),
                    )
                    nc.gpsimd.tensor_tensor(
                        ga[:, u0:u1, r2:re_, cs:ce],
                        ga[:, u0:u1, r2:re_, cs:ce],
                        tbP[:, u0:u1, :mr, :wc], op=acc,
                    )

            def ypred(u0, u1):
                # with A = L D L^T the forward pass leaves c = L^-1 b in col
                # 32 and u = L^-1 xtt in col 33; ypred = sum_k u_k c_k / d_k
                nc.vector.tensor_mul(
                    prod[:, u0:u1], ga[:, u0:u1, :, DP],
                    ga[:, u0:u1, :, DP + 1])
                nc.vector.tensor_tensor(
                    prod[:, u0:u1], prod[:, u0:u1], ga_diag[:, u0:u1],
                    op=mybir.AluOpType.divide,
                )
                nc.vector.tensor_reduce(
                    yp[:, u0:u1], prod[:, u0:u1],
                    mybir.AxisListType.X, mybir.AluOpType.add,
                )

            for slot in range(DP + OFFS[-1]):
                for u, (u0, u1) in enumerate(UNITS):
                    k = slot - OFFS[u]
                    if k < 0 or k > DP - 1:
                        continue
                    step(u0, u1, k)
                    if k == DP - 1:
                        ypred(u0, u1)

            nc.sync.dma_start(
                out_d.rearrange("(t p) one -> p (t one)", p=P), yp
            )

    nc.finalize()
    return nc


_cache: dict[float, object] = {}


def _get_nc(c2: float):
    if c2 not in _cache:
        _cache[c2] = _build_nc(c2)
    return _cache[c2]


def _build_xT(Xtrain, shard):
    """Pack [XtrT | XteT] with chunks at partition offsets 32g for 4-way
    row-group gram matmuls.  Row 31 of each group carries -|x|^2/2 on the
    train side and 1.0 on the test side, so the gram matmul computes
    S - sn/2 directly (no separate exp bias)."""
    out = np.zeros((P, 4 * P + TS), np.float32)
    XtrT = Xtrain.T
    nsn2 = -0.5 * np.sum(Xtrain * Xtrain, axis=1)       # [2048]
    for g in range(4):
        for cc in range(4):
            c = 4 * cc + g
            out[32 * g:32 * g + D, cc * P:(cc + 1) * P] = \
                XtrT[:, c * P:(c + 1) * P]
            out[32 * g + D, cc * P:(cc + 1) * P] = nsn2[c * P:(c + 1) * P]
        out[32 * g:32 * g + D, 4 * P:] = shard.T
        out[32 * g + D, 4 * P:] = 1.0
    return out


def _host_pack(Ytrain, Xtrain):
    """Train-side packing shared by all cores: the Z expansion as four
    e-strips of the upper outer-product pairs plus the x*y column."""
    Xt = np.concatenate(
        [np.ones((N_TRAIN, 1), np.float32), Xtrain], axis=1)  # [2048, 32]
    parts = []
    for s in range(4):
        rs = 8 * s + 8
        parts.append((Xt[:, :rs, None] * Xt[:, None, 8 * s:8 * s + 8])
                     .reshape(N_TRAIN, rs * 8))
    parts.append(Xt * Ytrain[:, 0:1])
    zz = np.concatenate(parts, axis=1)                  # [2048, 672]
    return np.ascontiguousarray(
        zz.reshape(NK, P, NZ).transpose(1, 0, 2).reshape(P, NK * NZ))


def _host_pack_test(shard, c2):
    """Test-side packing per core: ridge scale + design rows."""
    st = np.sum(shard * shard, axis=1)                  # [512]
    regt = np.ascontiguousarray(
        (REG * np.exp(0.5 * c2 * st)).reshape(NT, P).T.astype(np.float32))
    xtt = np.concatenate(
        [np.ones((TS, 1), np.float32), shard], axis=1)  # [512, 32]
    xtt = np.ascontiguousarray(
        xtt.reshape(NT, P, DP).transpose(1, 0, 2).reshape(P, NT * DP))
    return regt, xtt


def kernel(Ytrain, Xtrain, Xtest, log_lengthscale, _trace=False):
    Ytrain = np.ascontiguousarray(np.asarray(Ytrain, dtype=np.float32))
    Xtrain = np.ascontiguousarray(np.asarray(Xtrain, dtype=np.float32))
    Xtest = np.ascontiguousarray(np.asarray(Xtest, dtype=np.float32))
    lls = float(np.asarray(log_lengthscale, dtype=np.float32))
    c2 = float(np.exp(np.float32(-2.0 * lls)))

    nc = _get_nc(c2)
    zz = _host_pack(Ytrain, Xtrain)
    in_maps = []
    for core in range(NCORES):
        shard = np.ascontiguousarray(Xtest[core * TS:(core + 1) * TS])
        regt, xtt = _host_pack_test(shard, c2)
        in_maps.append({
            "xT": _build_xT(Xtrain, shard),
            "zz": zz,
            "regt": regt,
            "xtt": xtt,
        })
    res = run_bass_kernel_spmd(nc, in_maps, list(range(NCORES)),
                               trace=bool(_trace))
    outs = [np.asarray(res.results[c]["ypred"], dtype=np.float32)
            for c in range(NCORES)]
    full = np.concatenate(outs, axis=0)
    if _trace:
        return full, res
    return full


def _sim_in_map(inputs):
    """Core-0 input map for CoreSim timing (test.py helper)."""
    Ytrain = np.asarray(inputs["Ytrain"], dtype=np.float32)
    Xtrain = np.asarray(inputs["Xtrain"], dtype=np.float32)
    Xtest = np.asarray(inputs["Xtest"], dtype=np.float32)
    lls = float(np.asarray(inputs["log_lengthscale"], dtype=np.float32))
    c2 = float(np.exp(np.float32(-2.0 * lls)))
    shard = np.ascontiguousarray(Xtest[:TS])
    zz = _host_pack(Ytrain, Xtrain)
    regt, xtt = _host_pack_test(shard, c2)
    return c2, {
        "xT": _build_xT(Xtrain, shard),
        "zz": zz,
        "regt": regt,
        "xtt": xtt,
    }


# revision 25
# speedup vs baseline: 1.0517x; 1.0517x over previous
"""Trainium2 Bass kernel: batched locally-weighted ridge regression.

Per test point t: K[t,n] = exp(-|xte_t - xtr_n|^2 / (2 ls^2));
  A_t = Xtild^T diag(K[t]) Xtild + REG*I ; b_t = Xtild^T (K[t] * Y)
  ypred_t = xtild_t . A_t^{-1} b_t
Sharding: data-parallel over the 4096 test points -> 8 cores x 512.

On-device math uses a scaled kernel K'[t,n] = exp((S[n,t] - sn[n]/2) * c2)
(c2 = 1/ls^2), i.e. the exp(-st*c2/2) per-test factor is dropped; this
rescales A_t and b_t identically, so beta is preserved by using a
per-test ridge REG_t = REG * exp(st*c2/2).

Pipeline layout (per core; engines run concurrently):
  - Input DMA spread across SP/DVE/Pool/ACT queues (the cost model charges
    transfers to the issuing engine), zz ordered chunk-major so the XWX
    accumulation can chase the arrivals.
  - PE: warmup matmuls (clock ramp), then gram for the test-half 0,
    XWX for t-tile 0 (range-interleaved, chasing the exp chain on ACT),
    gram half 1, XWX t1..t3.  All matmuls f32r, 1 cycle/row (N>=256).
  - ACT: exp of the gram chunks at [P, 2, 256] grain, then the PSUM
    evacuations assembling [A | b | xtt] per t-tile.
  - Solve: LDLt-style symmetric elimination of 4 units of 128 systems,
    emitted with a staggered interleave so units pipeline.  Per step:
    Pool normalize_recip produces ps = pivot_row * (1/d) AND stores 1/d
    on the diagonal in one op; a Pool pivot-row lookahead (row k+1)
    keeps the serial chain engine-local; DVE updates a band of rows
    below the pivot as one rect; Pool covers the remaining rows with
    column-panel rectangles.  No back-substitution: with A = L D L^T,
    ypred = sum_k invp_k * (L^-1 b)_k * (L^-1 xtt)_k.
"""

import numpy as np

import concourse.bacc as bacc
import concourse.mybir as mybir
from concourse.bass import ts
from concourse.bass_utils import run_bass_kernel_spmd
from concourse.tile import TileContext

F32 = mybir.dt.float32
F32R = mybir.dt.float32r
P = 128
N_TRAIN = 2048
D = 31
DP = 32          # 1 + D
W = 34           # DP + two rhs columns: b (col 32), xtt (col 33)
N_TEST = 4096
NCORES = 8
TS = N_TEST // NCORES   # 512 test points per core
NT = TS // P            # 4 t-tiles
NK = N_TRAIN // P       # 16 train chunks
NZ = 672                # 4 e-strips of the upper pairs (64+128+192+256) + 32 (x*y)
REG = 1e-6
TH = 256                # test-half width for the gram/exp pipeline

N_WARMUP = 4            # PE clock-ramp warmup matmuls
UNITS = ((0, 1), (1, 2), (2, 3), (3, 4))
OFFS = (0, 13, 22, 31)
BSMAX = 16              # max DVE band rows

# engine cost model (ns) used to pick the DVE band size per step
_POOL_TAX, _POOL_EL = 8.0, 0.8333
_DVE_TAX, _DVE_EL = 90.0, 1.0417


def _panels_cost(r2: int, k: int, ns: int = 1) -> float:
    w_ = 0.0
    for p in range(r2 // 8, 4):
        re_ = 8 * p + 8
        if re_ <= r2:
            continue
        cs = max(8 * p, k + 1)
        ce = W if p == 3 else re_
        w_ += 2 * (_POOL_TAX + ns * (re_ - r2) * (ce - cs) * _POOL_EL)
    return w_


def _band(k: int, ns: int = 1) -> int:
    """DVE band rows at step k (rows k+2 .. k+2+bs); Pool panels cover the
    rest.  Chosen to balance the two engines' per-step busy time."""
    m1 = 30 - k
    if m1 <= 0:
        return 0
    w = W - 1 - k
    nrow = 2 if ns == 1 else 3
    base_pool = nrow * (_POOL_TAX + ns * w * _POOL_EL)
    best, best_cost = 0, None
    for bs in range(0, min(BSMAX, m1) + 1):
        dve = 2 * (_DVE_TAX + ns * bs * w * _DVE_EL) if bs else 0.0
        pool = base_pool + _panels_cost(k + 2 + bs, k, ns)
        cost = max(dve, pool)
        if best_cost is None or cost < best_cost:
            best, best_cost = bs, cost
    return best


def _build_nc(c2: float):
    """Build the single-core Bass program (SPMD across 8 cores)."""
    nc = bacc.Bacc(trn_type="TRN2")

    # transposed features packed for 4-way row-group gram matmuls:
    # [32g+d, cc*128+p] = Xtrain[(4*cc+g)*128+p, d]; cols 512: = XtestT x4
    xT_d = nc.dram_tensor("xT", [P, 4 * P + TS], F32R, kind="ExternalInput")
    zz_d = nc.dram_tensor("zz", [P, NK * NZ], F32R, kind="ExternalInput")
    regt_d = nc.dram_tensor("regt", [P, NT], F32, kind="ExternalInput")
    xtt_d = nc.dram_tensor("xtt", [P, NT * DP], F32, kind="ExternalInput")
    out_d = nc.dram_tensor("ypred", [TS, 1], F32, kind="ExternalOutput")

    with TileContext(nc) as tc:
        with (
            tc.tile_pool(name="sb", bufs=1) as sb,
            tc.tile_pool(name="pgram", bufs=2, space="PSUM") as pgram,
            tc.tile_pool(name="pxwx", bufs=4, space="PSUM") as pxwx,
        ):
            # ---- input loads spread across engine queues; chunk-major zz so
            # the XWX accumulation can chase the arrivals.
            xT = sb.tile([P, 4 * P + TS], F32R)
            zz = sb.tile([P, NK, NZ], F32R)
            zr = zz[:].rearrange("p c z -> p (c z)")
            regt = sb.tile([P, NT], F32)
            xtt = sb.tile([P, NT, DP], F32)

            def zslice(c):
                return (zr[:, ts(c, NZ)], zz_d[:, ts(c, NZ)])

            # SP: xT (needed first), xtt, zz odd chunks, regt
            nc.sync.dma_start(xT[:, 512:], xT_d[:, 512:])
            nc.sync.dma_start(xT[:, 0:P], xT_d[:, 0:P])
            nc.sync.dma_start(xT[:, P:512], xT_d[:, P:512])
            for c in (1, 3, 5, 7, 9, 11, 14):
                nc.sync.dma_start(*zslice(c))
            nc.sync.dma_start(
                xtt, xtt_d.rearrange("p (t d) -> p t d", t=NT)
            )
            nc.sync.dma_start(regt, regt_d[:, :])
            wu = sb.tile([P, 512], F32R)
            nc.vector.memset(wu, 1.0)
            # Pool: zz even chunks (Pool is solve-idle until ~10us)
            for c in (0, 2, 4, 6, 8, 10, 12):
                nc.gpsimd.dma_start(*zslice(c))
            # ga sub-diagonal zeroing on idle-early DVE (disjoint from the
            # evac strips, so it can run before the XWX finishes)
            # ACT: two slices in its idle window before the first gram
            # pair lands (~2.6us); exps are not delayed
            nc.scalar.dma_start(*zslice(13))
            nc.scalar.dma_start(*zslice(15))

            # ---- PE warmup: ramp the clock during the DMAs ----
            for _ in range(N_WARMUP):
                wps = pxwx.tile([P, 512], F32, tag="px")
                nc.tensor.matmul(wps, wu[0:8, 0:128], wu[0:8, :],
                                 start=True, stop=True)

            # ---- gram S' = S - sn/2 and K' = exp(S'*c2), by test-half;
            # the -sn/2 bias rides in row 31 of each packed xT group ----
            kp = sb.tile([P, NK, TS], F32R)

            def gram_pair(h, cp):
                sg = pgram.tile([P, 2, TH], F32, tag="sg")
                for i in range(2):
                    c = 2 * cp + i
                    cc, g = c // 4, c % 4
                    nc.tensor.matmul(
                        sg[:, i, :],
                        xT[32 * g:32 * g + DP, ts(cc, P)],
                        xT[32 * g:32 * g + DP,
                           4 * P + h * TH:4 * P + (h + 1) * TH],
                        start=True, stop=True,
                        tile_position=(32 * g, 0),
                    )
                nc.scalar.activation(
                    kp[:, 2 * cp:2 * cp + 2, h * TH:(h + 1) * TH], sg,
                    mybir.ActivationFunctionType.Exp,
                    scale=c2,
                )

            # ---- per t-tile: XWX/XWy matmuls (range-interleaved so both
            # PSUM accumulators chase the exp chain), then ACT evacuation ----
            ga = sb.tile([P, NT, DP, W], F32)
            ga_diag = ga[:].rearrange("p b r c -> p b (r c)")[:, :, ::W + 1]
            # zero the uninitialized sub-diagonal cells the band rects read
            # (row-block > col-block: not covered by any evac strip); runs on
            # idle-early DVE, disjoint from everything
            for h in range(NT):
                nc.vector.memset(ga[:, h, 8:32, 0:8], 0.0)
                nc.vector.memset(ga[:, h, 16:32, 8:16], 0.0)
                nc.vector.memset(ga[:, h, 24:32, 16:24], 0.0)

            def xwx_t(t, gram_b=False):
                pxa = pxwx.tile([P, 512], F32, tag="px")
                pxb = pxwx.tile([P, 512], F32, tag="px")
                corder = ((0, 1, 2, 3, 4, 6, 5, 8, 7, 10, 13, 9, 14, 15,
                           12, 11) if t == 0 else range(NK))
                for ci, c in enumerate(corder):
                    nc.tensor.matmul(
                        pxa[:, 0:384],
                        kp[:, c, ts(t, P)], zz[:, c, 0:384],
                        start=(ci == 0), stop=(ci == NK - 1),
                    )
                    nc.tensor.matmul(
                        pxb[:, 0:NZ - 384],
                        kp[:, c, ts(t, P)], zz[:, c, 384:NZ],
                        start=(ci == 0), stop=(ci == NK - 1),
                    )
                    if gram_b and ci % 2 == 1 and 5 <= ci <= 11:
                        # half-1 gram pairs soak the gate stalls of t0's
                        # accumulation; their exps queue behind exps-A on ACT
                        gram_pair(1, (ci - 5) // 2)
                # evacuation: t0 on (idle) DVE so the solve can start without
                # queuing behind ACT's exp chain; t1..t3 on ACT
                def ecopy(dst, src):
                    if t == 0:
                        nc.vector.tensor_scalar_mul(dst, src, 1.0)
                    else:
                        nc.scalar.copy(dst, src)
                off = 0
                for s in range(3):
                    rs = 8 * s + 8
                    ecopy(
                        ga[:, t, 0:rs, 8 * s:8 * s + 8],
                        pxa[:, off:off + 8 * rs].rearrange(
                            "p (r c) -> p r c", r=rs),
                    )
                    off += 8 * rs
                # strip 3 + rhs column + test design row from range B
                ecopy(
                    ga[:, t, :, 24:32],
                    pxb[:, 0:256].rearrange("p (r c) -> p r c", r=DP),
                )
                ecopy(ga[:, t, :, DP], pxb[:, 256:256 + DP])
                ecopy(ga[:, t, :, DP + 1], xtt[:, t])
                # per-test ridge on the diagonal (t0: DVE; rest: ACT)
                if t == 0:
                    nc.vector.tensor_add(
                        ga_diag[:, t], ga_diag[:, t],
                        regt[:, t, None].broadcast_to([P, DP]),
                    )
                else:
                    nc.scalar.add(ga_diag[:, t], ga_diag[:, t],
                                  regt[:, t:t + 1])

            for cp in range(NK // 2):
                gram_pair(0, cp)
            xwx_t(0, gram_b=True)
            for cp in range(4, NK // 2):
                gram_pair(1, cp)
            xwx_t(1)
            xwx_t(2)
            xwx_t(3)

            # ---- solve: symmetric (LDLt-style) elimination over UNITS of
            # 1-2 adjacent t-tiles (a multi-tile unit eliminates its tiles
            # in lockstep inside the same instructions: same op count, n x
            # the elements, so per-step taxes and serial chain amortize).
            # A[i,k] = A[k,i], so the update is upd[i,j] =
            # (A[k,i]/d)*A[k,j] = A[k,i]*ps[j]; only the upper triangle +
            # rhs is ever read.
            ps = sb.tile([P, NT, W], F32)
            yp = sb.tile([P, NT], F32)
            tbr = sb.tile([P, NT, W], F32)
            tbD = sb.tile([P, NT, BSMAX, W], F32)
            tbP = sb.tile([P, NT, DP, 12], F32)
            prod = sb.tile([P, NT, DP], F32)

            def step(u0, u1, k):
                if k == DP - 1:
                    return
                ns = u1 - u0
                w = W - 1 - k              # cols k+1..33 (incl rhs)
                # pivot: ps = -row_k[k+1:] / d  (negated so every downstream
                # update is an add; fused div+negate for 1-tile units)
                if ns == 1:
                    nc.gpsimd.tensor_scalar(
                        ps[:, u0, :w], ga[:, u0, k, k + 1:W],
                        ga[:, u0, k, k:k + 1], -1.0,
                        op0=mybir.AluOpType.divide,
                        op1=mybir.AluOpType.mult,
                    )
                    # lookahead row k+1, fused: ga_row += ps * A[k,k+1]
                    # (Pool-local chain: div -> row -> next div)
                    nc.gpsimd.scalar_tensor_tensor(
                        ga[:, u0, k + 1, k + 1:W], ps[:, u0, :w],
                        ga[:, u0, k, k + 1:k + 2], ga[:, u0, k + 1, k + 1:W],
                        op0=mybir.AluOpType.mult, op1=mybir.AluOpType.add,
                    )
                else:
                    # positive ps via broadcast divide; downstream ops
                    # subtract instead of add for this unit
                    nc.gpsimd.tensor_tensor(
                        ps[:, u0:u1, :w], ga[:, u0:u1, k, k + 1:W],
                        ga[:, u0:u1, k, k:k + 1].broadcast_to([P, ns, w]),
                        op=mybir.AluOpType.divide,
                    )
                    nc.gpsimd.tensor_mul(
                        tbr[:, u0:u1, :w], ps[:, u0:u1, :w],
                        ga[:, u0:u1, k, k + 1:k + 2]
                        .broadcast_to([P, ns, w]),
                    )
                    nc.gpsimd.tensor_sub(
                        ga[:, u0:u1, k + 1, k + 1:W],
                        ga[:, u0:u1, k + 1, k + 1:W], tbr[:, u0:u1, :w])
                acc = (mybir.AluOpType.add if ns == 1
                       else mybir.AluOpType.subtract)
                bs = _band(k, ns)
                r2 = k + 2 + bs
                if bs > 0:
                    # DVE band: rows k+2..r2, all cols k+1..33
                    nc.vector.tensor_mul(
                        tbD[:, u0:u1, :bs, :w],
                        ga[:, u0:u1, k, k + 2:r2, None]
                        .broadcast_to([P, ns, bs, w]),
                        ps[:, u0:u1, None, :w].broadcast_to([P, ns, bs, w]),
                    )
                    nc.vector.tensor_tensor(
                        ga[:, u0:u1, k + 2:r2, k + 1:W],
                        ga[:, u0:u1, k + 2:r2, k + 1:W],
                        tbD[:, u0:u1, :bs, :w], op=acc,
                    )
                # remaining rows r2..31 by column panels (Pool)
                for p in range(r2 // 8, 4):
                    re_ = 8 * p + 8
                    if re_ <= r2:
                        continue
                    cs = max(8 * p, k + 1)
                    ce = W if p == 3 else re_
                    mr, wc = re_ - r2, ce - cs
                    nc.gpsimd.tensor_mul(
                        tbP[:, u0:u1, :mr, :wc],
                        ga[:, u0:u1, k, r2:re_, None]
                        .broadcast_to([P, ns, mr, wc]),
                        ps[:, u0:u1, None, cs - k - 1:ce - k - 1]
                        .broadcast_to([P, ns, mr, wc]),
                    )
                    nc.gpsimd.tensor_tensor(
                        ga[:, u0:u1, r2:re_, cs:ce],
                        ga[:, u0:u1, r2:re_, cs:ce],
                        tbP[:, u0:u1, :mr, :wc], op=acc,
                    )

            def ypred(u0, u1):
                # with A = L D L^T the forward pass leaves c = L^-1 b in col
                # 32 and u = L^-1 xtt in col 33; ypred = sum_k u_k c_k / d_k
                nc.vector.tensor_mul(
                    prod[:, u0:u1], ga[:, u0:u1, :, DP],
                    ga[:, u0:u1, :, DP + 1])
                nc.vector.tensor_tensor(
                    prod[:, u0:u1], prod[:, u0:u1], ga_diag[:, u0:u1],
                    op=mybir.AluOpType.divide,
                )
                nc.vector.tensor_reduce(
                    yp[:, u0:u1], prod[:, u0:u1],
                    mybir.AxisListType.X, mybir.AluOpType.add,
                )

            for slot in range(DP + OFFS[-1]):
                for u, (u0, u1) in enumerate(UNITS):
                    k = slot - OFFS[u]
                    if k < 0 or k > DP - 1:
                        continue
                    step(u0, u1, k)
                    if k == DP - 1:
                        ypred(u0, u1)

            nc.sync.dma_start(
                out_d.rearrange("(t p) one -> p (t one)", p=P), yp
            )

    nc.finalize()
    return nc


_cache: dict[float, object] = {}


def _get_nc(c2: float):
    if c2 not in _cache:
        _cache[c2] = _build_nc(c2)
    return _cache[c2]


def _build_xT(Xtrain, shard):
    """Pack [XtrT | XteT] with chunks at partition offsets 32g for 4-way
    row-group gram matmuls.  Row 31 of each group carries -|x|^2/2 on the
    train side and 1.0 on the test side, so the gram matmul computes
    S - sn/2 directly (no separate exp bias)."""
    out = np.zeros((P, 4 * P + TS), np.float32)
    XtrT = Xtrain.T
    nsn2 = -0.5 * np.sum(Xtrain * Xtrain, axis=1)       # [2048]
    for g in range(4):
        for cc in range(4):
            c = 4 * cc + g
            out[32 * g:32 * g + D, cc * P:(cc + 1) * P] = \
                XtrT[:, c * P:(c + 1) * P]
            out[32 * g + D, cc * P:(cc + 1) * P] = nsn2[c * P:(c + 1) * P]
        out[32 * g:32 * g + D, 4 * P:] = shard.T
        out[32 * g + D, 4 * P:] = 1.0
    return out


def _host_pack(Ytrain, Xtrain):
    """Train-side packing shared by all cores: the Z expansion as four
    e-strips of the upper outer-product pairs plus the x*y column."""
    Xt = np.concatenate(
        [np.ones((N_TRAIN, 1), np.float32), Xtrain], axis=1)  # [2048, 32]
    parts = []
    for s in range(4):
        rs = 8 * s + 8
        parts.append((Xt[:, :rs, None] * Xt[:, None, 8 * s:8 * s + 8])
                     .reshape(N_TRAIN, rs * 8))
    parts.append(Xt * Ytrain[:, 0:1])
    zz = np.concatenate(parts, axis=1)                  # [2048, 672]
    return np.ascontiguousarray(
        zz.reshape(NK, P, NZ).transpose(1, 0, 2).reshape(P, NK * NZ))


def _host_pack_test(shard, c2):
    """Test-side packing per core: ridge scale + design rows."""
    st = np.sum(shard * shard, axis=1)                  # [512]
    regt = np.ascontiguousarray(
        (REG * np.exp(0.5 * c2 * st)).reshape(NT, P).T.astype(np.float32))
    xtt = np.concatenate(
        [np.ones((TS, 1), np.float32), shard], axis=1)  # [512, 32]
    xtt = np.ascontiguousarray(
        xtt.reshape(NT, P, DP).transpose(1, 0, 2).reshape(P, NT * DP))
    return regt, xtt


def kernel(Ytrain, Xtrain, Xtest, log_lengthscale, _trace=False):
    Ytrain = np.ascontiguousarray(np.asarray(Ytrain, dtype=np.float32))
    Xtrain = np.ascontiguousarray(np.asarray(Xtrain, dtype=np.float32))
    Xtest = np.ascontiguousarray(np.asarray(Xtest, dtype=np.float32))
    lls = float(np.asarray(log_lengthscale, dtype=np.float32))
    c2 = float(np.exp(np.float32(-2.0 * lls)))

    nc = _get_nc(c2)
    zz = _host_pack(Ytrain, Xtrain)
    in_maps = []
    for core in range(NCORES):
        shard = np.ascontiguousarray(Xtest[core * TS:(core + 1) * TS])
        regt, xtt = _host_pack_test(shard, c2)
        in_maps.append({
            "xT": _build_xT(Xtrain, shard),
            "zz": zz,
            "regt": regt,
            "xtt": xtt,
        })
    res = run_bass_kernel_spmd(nc, in_maps, list(range(NCORES)),
                               trace=bool(_trace))
    outs = [np.asarray(res.results[c]["ypred"], dtype=np.float32)
            for c in range(NCORES)]
    full = np.concatenate(outs, axis=0)
    if _trace:
        return full, res
    return full


def _sim_in_map(inputs):
    """Core-0 input map for CoreSim timing (test.py helper)."""
    Ytrain = np.asarray(inputs["Ytrain"], dtype=np.float32)
    Xtrain = np.asarray(inputs["Xtrain"], dtype=np.float32)
    Xtest = np.asarray(inputs["Xtest"], dtype=np.float32)
    lls = float(np.asarray(inputs["log_lengthscale"], dtype=np.float32))
    c2 = float(np.exp(np.float32(-2.0 * lls)))
    shard = np.ascontiguousarray(Xtest[:TS])
    zz = _host_pack(Ytrain, Xtrain)
    regt, xtt = _host_pack_test(shard, c2)
    return c2, {
        "xT": _build_xT(Xtrain, shard),
        "zz": zz,
        "regt": regt,
        "xtt": xtt,
    }
